# revision 24
# baseline (speedup 1.0000x reference)
"""TRN2 Bass kernel: 2-layer bidirectional LSTM encoder (nn_BiLstmCellEncoder).

Full-input contract: kernel(**inputs) takes the unsharded inputs of
reference.setup_inputs() and returns the full [128, 200, 1024] fp32 output.

Sharding: cross-core layer pipelining at batch 64. The 8 cores form 4 pairs
[[0,1],[2,3],[4,5],[6,7]]; each pair owns one (direction, batch-half) slice.
The even core runs the layer-0 recurrence, the odd core runs layer 1 lagged
by LAG blocks, consuming the even core's masked h0 blocks streamed through a
per-iteration 2-rank AllGather (DRAM bounce tiles). One chain per core keeps
the per-step LDWEIGHTS volume at 72 tiles (vs 144 for two on-core layers) —
the PE is weight-load-bandwidth bound, and batch 64 rides free in the moving
operand.

Gate rows are host-permuted to [i, g, f, o] so the i+g gate adds merge into
one DVE op reading contiguous psum/gx slices. Step order: i,g matmuls (their
tanh/sigmoid chain starts at 50% of the burst), then f (cell-state update
runs under the o matmuls), then o. The next block's gx matmuls (N=512
streams) are woven one per 8 recurrence matmuls: each long stream gives the
PE's reorder window room to pull the next LDWEIGHTS ahead (FWL/ldw-opt is
unavailable in this toolchain, so the 64 per-step weight reloads otherwise
serialize); measured -7.5% vs producing gx in a tail burst. The mask
multiply runs on GPSIMD (Pool) to stay off the DVE critical FIFO. win(k) is
gathered at the end of iteration k and staged as gaths[m-LAG], giving the
2-rank AllGather ~2 blocks of slack (its serial latency is ~100us; LAG=2
measured slower because that slack goes to ~0).

Role asymmetry lives entirely in host-prepared data (same SPMD program):
L1 cores get zero xT (input arrives via the collective; st = A + B*sel with
a per-core 0/1 constant), a per-block bias table zeroed for the first LAG
blocks (zero-input pre-chain steps then keep h=c exactly 0), and masks
shifted by LAG blocks. Even cores' final-output writes land in slots the
host ignores.
"""
import sys
sys.path.insert(0, '/opt/trn_rl_repo')

import numpy as np
import ml_dtypes

import concourse.bass as bass
import concourse.mybir as mybir
from concourse import bacc
import concourse.tile as tile
from concourse import bass_utils

F32 = mybir.dt.float32
BF16 = mybir.dt.bfloat16
AF = mybir.ActivationFunctionType
OP = mybir.AluOpType

B, S, D, H = 128, 200, 512, 512
NG = 4 * H
KC = 4     # 128-row chunks over H/D (contraction)
MC = 16    # 128-row chunks over the 4H gate rows
BC = 64    # batch per core (one half)
TB = 8     # time-steps per pipeline block
NB = S // TB          # real blocks (25)
LAG = 3               # L1 lags L0 by LAG blocks (cc + stage + gx pipeline)
NI = NB + LAG         # program iterations (28)
SP = NI * TB          # padded t-slots (224)
NCORES = 8
GROUPS = [[0, 1], [2, 3], [4, 5], [6, 7]]

TRACE = False
NOCC = False
LAST_RESULTS = None


def _build():
    nc = bacc.Bacc("TRN2", target_bir_lowering=False, debug=False,
                   num_devices=NCORES)

    xT = nc.dram_tensor("xT", [KC, 128, SP, BC], BF16, kind="ExternalInput")
    wih = nc.dram_tensor("wihT", [KC, 128, NG], BF16, kind="ExternalInput")
    whh = nc.dram_tensor("whhT", [KC, 128, NG], BF16, kind="ExternalInput")
    biasblk = nc.dram_tensor("biasblk", [128, NI, MC], F32,
                             kind="ExternalInput")
    maskrep = nc.dram_tensor("maskrep", [128, SP, BC], BF16,
                             kind="ExternalInput")
    selB = nc.dram_tensor("selB", [128, TB, BC], BF16, kind="ExternalInput")
    out = nc.dram_tensor("out", [KC, 128, S, BC], BF16,
                         kind="ExternalOutput")

    with tile.TileContext(nc) as tc:
        with tc.tile_pool(name="const", bufs=1) as cpool, \
             tc.tile_pool(name="state", bufs=1) as spool, \
             tc.tile_pool(name="stage", bufs=3) as stpool, \
             tc.tile_pool(name="gx", bufs=2) as gxpool, \
             tc.tile_pool(name="win", bufs=3) as wpool, \
             tc.tile_pool(name="act", bufs=8) as apool, \
             tc.tile_pool(name="dram", bufs=1, space="DRAM") as dpool, \
             tc.tile_pool(name="psg", bufs=2, space="PSUM") as psg_pool, \
             tc.tile_pool(name="psp", bufs=3, space="PSUM") as psp_pool:

            wih_sb = cpool.tile([128, KC, NG], BF16, tag="wih")
            whh_sb = cpool.tile([128, KC, NG], BF16, tag="whh")
            bias_sb = cpool.tile([128, NI, MC], F32, tag="bias")
            sel_sb = cpool.tile([128, TB, BC], BF16, tag="sel")
            nc.sync.dma_start(wih_sb[:], wih.ap().rearrange("k p c -> p k c"))
            nc.sync.dma_start(whh_sb[:], whh.ap().rearrange("k p c -> p k c"))
            nc.sync.dma_start(bias_sb[:], biasblk.ap())
            nc.sync.dma_start(sel_sb[:], selB.ap())

            h_sb = spool.tile([128, KC, BC], BF16, tag="h")
            c_sb = spool.tile([128, KC, BC], F32, tag="c")
            nc.vector.memset(h_sb[:], 0.0)
            nc.vector.memset(c_sb[:], 0.0)

            contribs = [dpool.tile([KC, 128, TB, BC], BF16, tag=f"ctb{k}",
                                   name=f"ctb{k}") for k in range(NI)]
            gaths = [dpool.tile([2 * KC, 128, TB, BC], BF16, tag=f"gth{k}",
                                name=f"gth{k}") for k in range(NI - LAG)]

            ctx = {}

            def stage_select(m):
                """Stage st_m = xT[m] + gathered_{m-LAG}[rank0] * sel."""
                a = stpool.tile([128, KC, TB, BC], BF16, tag="stA", name="A")
                nc.sync.dma_start(
                    a[:], xT.ap()[:, :, m * TB:(m + 1) * TB, :]
                    .rearrange("k p t b -> p k t b"))
                if m < LAG:
                    # no gather exists yet; L1's xT is zeros, so a works
                    # for both roles
                    st = a
                else:
                    st = stpool.tile([128, KC, TB, BC], BF16, tag="stS",
                                     name="St")
                    b = stpool.tile([128, KC, TB, BC], BF16, tag="stB",
                                    name="Bt")
                    if NOCC:
                        nc.sync.dma_start(
                            b[:], contribs[m - LAG][:]
                            .rearrange("k p t b -> p k t b"))
                    else:
                        nc.sync.dma_start(
                            b[:], gaths[m - LAG][0:KC, :, :, :]
                            .rearrange("k p t b -> p k t b"))
                    for kc in range(KC):
                        nc.vector.tensor_tensor(b[:, kc], b[:, kc], sel_sb[:],
                                                OP.mult)
                    nc.vector.tensor_tensor(st[:], a[:], b[:], OP.add)
                msk = stpool.tile([128, TB, BC], BF16, tag="msk", name="mk")
                nc.sync.dma_start(msk[:],
                                  maskrep.ap()[:, m * TB:(m + 1) * TB, :])
                return st, msk

            def produce(m, st, gx, mm):
                pp = psp_pool.tile([128, TB, BC], F32, tag="psp", name="pp")
                for kc in range(KC):
                    nc.tensor.matmul(
                        pp[:],
                        wih_sb[:, kc, mm * 128:(mm + 1) * 128],
                        st[:, kc, :, :],
                        start=(kc == 0), stop=(kc == KC - 1))
                nc.scalar.activation(gx[:, :, mm, :], pp[:], AF.Identity,
                                     bias=bias_sb[:, m, mm:mm + 1])

            def emit_step(k, j):
                """Permuted gate order i(0:4), g(4:8), f(8:12), o(12:16).
                i,g matmuls first (pgig), then f (pgfo 0:4), then o
                (pgfo 4:8); the i+g adds merge into one DVE op. The next
                block's gx matmuls are woven one-per-8 rec matmuls so their
                N=512 streams hide the rec LDWEIGHTS pull-aheads."""
                gx_cur, win, msk = ctx['gx_cur'], ctx['win'], ctx['msk']
                prod = []
                if k + 1 < NI:
                    for mm in range(j * 2, j * 2 + 2):
                        pp = psp_pool.tile([128, TB, BC], F32, tag="psp",
                                           name="pp")
                        for kc in range(KC):
                            prod.append((mm, kc, pp))
                pstate = {'i': 0}

                def weave():
                    if pstate['i'] >= len(prod):
                        return
                    mm, kc, pp = prod[pstate['i']]
                    pstate['i'] += 1
                    nc.tensor.matmul(
                        pp[:],
                        wih_sb[:, kc, mm * 128:(mm + 1) * 128],
                        ctx['st_next'][:, kc, :, :],
                        start=(kc == 0), stop=(kc == KC - 1))
                    if kc == KC - 1:
                        nc.scalar.activation(
                            ctx['gx_next'][:, :, mm, :], pp[:], AF.Identity,
                            bias=bias_sb[:, k + 1, mm:mm + 1])

                pgig = psg_pool.tile([128, 8, BC], F32, tag="psgig",
                                     name="pgig", bufs=3)
                pgfo = psg_pool.tile([128, 8, BC], F32, tag="psgfo",
                                     name="pgfo")
                for mg in range(8):      # i (0-3), g (4-7)
                    for kc in range(KC):
                        nc.tensor.matmul(
                            pgig[:, mg, :],
                            whh_sb[:, kc, mg * 128:(mg + 1) * 128],
                            h_sb[:, kc, :],
                            start=(kc == 0), stop=(kc == KC - 1))
                    if mg % 2 == 1:
                        weave()
                # i,g psum complete at 50% of the burst: merged add, then
                # sigmoid/tanh/i*g under the f,o matmul stream
                gsig = apool.tile([128, 8, BC], F32, tag="gsig", name="gsig")
                nc.vector.tensor_tensor(gsig[:], pgig[:],
                                        gx_cur[:, j, 0:8, :], OP.add)
                s_i = apool.tile([128, 4, BC], F32, tag="s_i", name="s_i")
                nc.scalar.activation(s_i[:], gsig[:, 0:4, :], AF.Sigmoid)
                t_g = apool.tile([128, 4, BC], F32, tag="t_g", name="t_g")
                nc.scalar.activation(t_g[:], gsig[:, 4:8, :], AF.Tanh)
                tmp = apool.tile([128, 4, BC], F32, tag="tmp", name="tmp")
                nc.vector.tensor_tensor(tmp[:], s_i[:], t_g[:], OP.mult)
                for mg in range(8, 12):  # f
                    for kc in range(KC):
                        nc.tensor.matmul(
                            pgfo[:, mg - 8, :],
                            whh_sb[:, kc, mg * 128:(mg + 1) * 128],
                            h_sb[:, kc, :],
                            start=(kc == 0), stop=(kc == KC - 1))
                    if mg % 2 == 1:
                        weave()
                gsf = apool.tile([128, 4, BC], F32, tag="gsf", name="gsf")
                nc.vector.tensor_tensor(gsf[:], pgfo[:, 0:4, :],
                                        gx_cur[:, j, 8:12, :], OP.add)
                s_f = apool.tile([128, 4, BC], F32, tag="s_f", name="s_f")
                nc.scalar.activation(s_f[:], gsf[:], AF.Sigmoid)
                nc.vector.tensor_tensor(c_sb[:], c_sb[:], s_f[:], OP.mult)
                nc.vector.tensor_tensor(c_sb[:], c_sb[:], tmp[:], OP.add)
                tc_t = apool.tile([128, 4, BC], F32, tag="tc", name="tc_t")
                nc.scalar.activation(tc_t[:], c_sb[:], AF.Tanh)
                for mg in range(12, 16):  # o
                    for kc in range(KC):
                        nc.tensor.matmul(
                            pgfo[:, mg - 8, :],
                            whh_sb[:, kc, mg * 128:(mg + 1) * 128],
                            h_sb[:, kc, :],
                            start=(kc == 0), stop=(kc == KC - 1))
                    if mg % 2 == 1:
                        weave()
                gso = apool.tile([128, 4, BC], F32, tag="gso", name="gso")
                nc.vector.tensor_tensor(gso[:], pgfo[:, 4:8, :],
                                        gx_cur[:, j, 12:16, :], OP.add)
                s_o = apool.tile([128, 4, BC], F32, tag="s_o", name="s_o")
                nc.scalar.activation(s_o[:], gso[:], AF.Sigmoid)
                nc.vector.tensor_tensor(h_sb[:], s_o[:], tc_t[:], OP.mult)
                nc.gpsimd.tensor_tensor(
                    win[:, :, j, :], h_sb[:],
                    msk[:, j, None, :].to_broadcast([128, KC, BC]),
                    OP.mult)
                while pstate['i'] < len(prod):
                    weave()

            # prelude: block 0 staged + gx fully produced up front
            st0, msk0 = stage_select(0)
            gx0 = gxpool.tile([128, TB, MC, BC], BF16, tag="gx", name="gx0")
            for mm in range(MC):
                produce(0, st0, gx0, mm)
            ctx['gx_cur'], ctx['msk'] = gx0, msk0

            for k in range(NI):
                if k + 1 < NI:
                    ctx['st_next'], ctx['msk_next'] = stage_select(k + 1)
                    ctx['gx_next'] = gxpool.tile([128, TB, MC, BC], BF16,
                                                 tag="gx", name="gxn")
                ctx['win'] = wpool.tile([128, KC, TB, BC], BF16, tag="win",
                                        name="win")
                for j in range(TB):
                    emit_step(k, j)
                if k < NI - LAG:
                    nc.sync.dma_start(
                        contribs[k][:].rearrange("k p t b -> p k t b"),
                        ctx['win'][:])
                    if not NOCC:
                        nc.gpsimd.collective_compute(
                            "AllGather", OP.bypass, replica_groups=GROUPS,
                            ins=[contribs[k].opt()], outs=[gaths[k].opt()])
                if k >= LAG:
                    nc.sync.dma_start(
                        out.ap()[:, :, (k - LAG) * TB:(k - LAG + 1) * TB, :]
                        .rearrange("k p t b -> p k t b"),
                        ctx['win'][:])
                if k + 1 < NI:
                    ctx['gx_cur'] = ctx['gx_next']
                    ctx['msk'] = ctx['msk_next']
    nc.compile()
    return nc


_NC = None


def _get_nc():
    global _NC
    if _NC is None:
        _NC = _build()
    return _NC


# host-side gate-row permutation: [i, g, f, o] (original PyTorch i,f,g,o)
_PERM = np.concatenate([np.arange(0, 512), np.arange(1024, 1536),
                        np.arange(512, 1024), np.arange(1536, 2048)])


def _prep_in_maps(x, lens, Wih_f, Whh_f, bih_f, bhh_f, Wih_b, Whh_b,
                  bih_b, bhh_b):
    bf = ml_dtypes.bfloat16
    x = np.asarray(x, dtype=np.float32)
    lens_np = np.asarray(lens).astype(np.int64)
    valid_full = (np.arange(S)[None, :] < lens_np[:, None]).astype(np.float32)

    Ws = {0: (np.asarray(Wih_f), np.asarray(Whh_f),
              np.asarray(bih_f), np.asarray(bhh_f)),
          1: (np.asarray(Wih_b), np.asarray(Whh_b),
              np.asarray(bih_b), np.asarray(bhh_b))}

    in_maps = []
    for c in range(NCORES):
        pair, role = c // 2, c % 2       # role 0 = layer 0, 1 = layer 1
        dirn, half = pair // 2, pair % 2
        bsl = slice(half * BC, (half + 1) * BC)
        valid = valid_full[bsl]
        xs = x[bsl]
        if dirn == 1:
            xs = xs[:, ::-1]
            valid = valid[:, ::-1]
        Wihs, Whhs, bihs, bhhs = Ws[dirn]
        lyr = role

        xpad = np.zeros((BC, SP, D), np.float32)
        if role == 0:
            xpad[:, :S] = xs

        mask = np.zeros((SP, BC), np.float32)
        off = 0 if role == 0 else LAG * TB
        mask[off:off + S] = valid.T

        bias_real = (bihs[lyr] + bhhs[lyr]).astype(np.float32)[_PERM]
        bb = np.zeros((NI, MC, 128), np.float32)
        kmin = 0 if role == 0 else LAG
        bb[kmin:] = bias_real.reshape(MC, 128)[None]

        m = {
            "xT": np.ascontiguousarray(xpad.transpose(2, 1, 0)).reshape(
                KC, 128, SP, BC).astype(bf),
            "maskrep": np.broadcast_to(mask[None], (128, SP, BC)).astype(bf)
                .copy(),
            "selB": np.full((128, TB, BC), float(role), bf),
            "wihT": np.ascontiguousarray(Wihs[lyr][_PERM].T).reshape(
                KC, 128, NG).astype(bf),
            "whhT": np.ascontiguousarray(Whhs[lyr][_PERM].T).reshape(
                KC, 128, NG).astype(bf),
            "biasblk": np.ascontiguousarray(bb.transpose(2, 0, 1)),
        }
        in_maps.append(m)
    return in_maps


def _assemble(results):
    outp = np.empty((B, S, 2 * H), dtype=np.float32)
    for pair in range(4):
        dirn, half = pair // 2, pair % 2
        c = pair * 2 + 1                  # odd core = layer-1 output
        arr = results[c]["out"].astype(np.float32).reshape(H, S, BC)\
            .transpose(2, 1, 0)
        if dirn == 1:
            arr = arr[:, ::-1, :]
        outp[half * BC:(half + 1) * BC, :, dirn * H:(dirn + 1) * H] = arr
    return outp


def kernel(x, lens, Wih_f, Whh_f, bih_f, bhh_f, Wih_b, Whh_b, bih_b, bhh_b):
    global LAST_RESULTS
    in_maps = _prep_in_maps(x, lens, Wih_f, Whh_f, bih_f, bhh_f,
                            Wih_b, Whh_b, bih_b, bhh_b)
    nc = _get_nc()
    res = bass_utils.run_bass_kernel_spmd(nc, in_maps, list(range(NCORES)),
                                          trace=TRACE)
    LAST_RESULTS = res
    return _assemble(res.results)

